# revision 1
# baseline (speedup 1.0000x reference)
"""Multi-head causal attention kernel for Trainium2 (8 NeuronCores).

Problem: B=4, S=2048, HID=1024, H=16 heads (head_dim 64), causal mask,
fp32 I/O.  out = softmax(mask + (XqWq)(XkWk)^T/8) (XvWv) Wo

Sharding: 8 cores = 4 batches x 2 head-groups.  Core c handles batch
c//2 and heads (c%2)*8 .. +8 (dk slice of 512).  Each core computes a
full-shape [S, HID] partial output (its head-group's contribution
through Wo); the host sums the two partials per batch.

Per-core dataflow (all matmuls in float32r = TF32-like, full PE rate):
  - PE-transpose X chunks -> X^T; project to kT (persistent, [e,s]
    layout, 2 heads per 128-partition tile), qT (rotating per-512-q
    window) and v (persistent, natural [s,e] with a ones column per
    head so the PV matmul also emits softmax denominators).
  - Attention in transposed [k,q] orientation per (q-window j, head
    pair): logits^T = kT-chunk (stationary) x qT (moving) with
    causally-restricted columns; additive triangular mask on diagonal
    blocks (DVE, in PSUM); exp on ScalarE PSUM->SBUF; PV accumulates
    ctx^T in PSUM (per-element has_written makes partial-range
    accumulation correct).  Denominator row -> reciprocal (DVE) ->
    partition_broadcast (GpSimd) -> multiply-evacuate ctx^T (DVE).
  - Output projection ctx^T.T @ Wo per q-window, fused into the stream.

The projection work for q-window j+1 is emitted interleaved with the
attention work of window j: the projection matmuls act as PE filler
that keeps the PE HAM activity monitor busy (otherwise the exp-bound
attention inner loop lets the PE clock-gate down to 1.2 GHz).
"""

import numpy as np

B, S, HID = 4, 2048, 1024
H_LOCAL, E_LOCAL = 8, 512  # heads / dk columns handled per core
N_CORES = 8
USE_F32R = True

_cached = {}


def _build():
    from concourse import bacc
    import concourse.bass as bass
    import concourse.mybir as mybir
    import concourse.tile as tile
    from concourse.masks import make_identity

    F32 = mybir.dt.float32
    F32R = mybir.dt.float32r if USE_F32R else mybir.dt.float32
    Exp = mybir.ActivationFunctionType.Exp

    nc = bacc.Bacc()
    xq = nc.dram_tensor("xq", [S, HID], F32R, kind="ExternalInput")
    xk = nc.dram_tensor("xk", [S, HID], F32R, kind="ExternalInput")
    xv = nc.dram_tensor("xv", [S, HID], F32R, kind="ExternalInput")
    wq = nc.dram_tensor("wq", [HID, E_LOCAL], F32R, kind="ExternalInput")
    wk = nc.dram_tensor("wk", [HID, E_LOCAL], F32R, kind="ExternalInput")
    wv = nc.dram_tensor("wv", [HID, E_LOCAL], F32R, kind="ExternalInput")
    wo = nc.dram_tensor("wo", [E_LOCAL, HID], F32R, kind="ExternalInput")
    out = nc.dram_tensor("out", [S, HID], F32, kind="ExternalOutput")

    NST = 8           # projection s-tiles
    STW = S // NST    # 256 rows per s-tile
    NSC = STW // 128  # 2 s-chunks per s-tile
    NDC = HID // 128  # 8 d-chunks
    NEC = E_LOCAL // 128  # 4 e-chunks = head pairs
    NKC = S // 128    # 16 k-chunks
    NQT = 4           # q windows of 512

    with tile.TileContext(nc) as tc:
        with (
            tc.sbuf_pool(name="consts", bufs=1) as consts,
            tc.sbuf_pool(name="persist", bufs=1) as persist,
            tc.sbuf_pool(name="stream", bufs=1) as sm,
            tc.psum_pool(name="ps", bufs=1) as ps,
        ):
            ident_f = consts.tile([128, 128], F32)
            make_identity(nc, ident_f)
            ident = consts.tile([128, 128], F32R)
            nc.vector.tensor_copy(ident, ident_f)
            # additive causal mask for diagonal [k,q] blocks: 0 where
            # k <= q else -1e9
            trimask = consts.tile([128, 128], F32)
            nc.gpsimd.memset(trimask, 0.0)
            nc.gpsimd.affine_select(
                out=trimask, in_=trimask,
                compare_op=mybir.AluOpType.is_ge, fill=-1e9, base=0,
                pattern=[[1, 128]], channel_multiplier=-1,
            )
            ones_col = consts.tile([128, 1], F32)
            nc.vector.memset(ones_col, 1.0)

            kt_sb = [persist.tile([128, S], F32R, name=f"kt{i}", tag=f"kt{i}")
                     for i in range(NEC)]
            v_sb = [persist.tile([128, H_LOCAL, 65], F32R, name=f"v{i}",
                                 tag=f"v{i}") for i in range(NKC)]

            wq_sb = sm.tile([128, NDC, E_LOCAL], F32R, tag="wq", bufs=1)
            wk_sb = sm.tile([128, NDC, E_LOCAL], F32R, tag="wk", bufs=1)
            wv_sb = sm.tile([128, NDC, E_LOCAL], F32R, tag="wv", bufs=1)
            wo_sb = sm.tile([128, NEC, HID], F32R, tag="wo", bufs=1)
            nc.sync.dma_start(
                out=wq_sb, in_=wq.rearrange("(dc p) e -> p dc e", p=128))
            nc.sync.dma_start(
                out=wk_sb, in_=wk.rearrange("(dc p) e -> p dc e", p=128))
            nc.sync.dma_start(
                out=wv_sb, in_=wv.rearrange("(dc p) e -> p dc e", p=128))
            nc.sync.dma_start(
                out=wo_sb, in_=wo.rearrange("(dv p) n -> p dv n", p=128))

            qt_rot = {}   # (window, ec) -> [128, 512] tile
            ctx_rot = {}  # (window, hp) -> [128, 512] tile

            def proj_unit(st, tname):
                """Load + transpose + project one input tensor s-tile."""
                s0 = st * STW
                w = st // 2
                xdram = {"q": xq, "k": xk, "v": xv}[tname]
                xnat = sm.tile([128, NSC, HID], F32R, tag="xnat", bufs=2,
                               name=f"xnat_{tname}{st}")
                nc.sync.dma_start(
                    out=xnat,
                    in_=xdram[s0:s0 + STW, :].rearrange(
                        "(sc p) d -> p sc d", p=128))
                xt = sm.tile([128, NDC, STW], F32R, tag="xt", bufs=2,
                             name=f"xt_{tname}{st}")
                for dcp in range(NDC // 2):
                    tp = ps.tile([128, 512], F32R, tag="work", bufs=2,
                                 name=f"tp_{tname}{st}_{dcp}")
                    for k2 in range(2):
                        dc = dcp * 2 + k2
                        for sc in range(NSC):
                            nc.tensor.transpose(
                                tp[:, k2 * STW + sc * 128:
                                   k2 * STW + (sc + 1) * 128],
                                xnat[:, sc, dc * 128:(dc + 1) * 128],
                                ident)
                    nc.vector.tensor_copy(xt[:, dcp * 2:dcp * 2 + 2, :], tp)

                if tname == "q":
                    for ec in range(NEC):
                        if st % 2 == 0:
                            qt_rot[(w, ec)] = sm.tile(
                                [128, 512], F32R, tag=f"qtr{ec}", bufs=2,
                                name=f"qtr{ec}_{w}")
                        pj = ps.tile([128, STW], F32, tag="work", bufs=2,
                                     name=f"pjq_{st}_{ec}")
                        for dc in range(NDC):
                            nc.tensor.matmul(
                                pj, wq_sb[:, dc, ec * 128:(ec + 1) * 128],
                                xt[:, dc, :],
                                start=(dc == 0), stop=(dc == NDC - 1))
                        off = (st % 2) * STW
                        nc.vector.tensor_copy(
                            qt_rot[(w, ec)][:, off:off + STW], pj)
                elif tname == "k":
                    for ec in range(NEC):
                        pj = ps.tile([128, STW], F32, tag="work", bufs=2,
                                     name=f"pjk_{st}_{ec}")
                        for dc in range(NDC):
                            nc.tensor.matmul(
                                pj, wk_sb[:, dc, ec * 128:(ec + 1) * 128],
                                xt[:, dc, :],
                                start=(dc == 0), stop=(dc == NDC - 1))
                        nc.vector.tensor_copy(
                            kt_sb[ec][:, s0:s0 + STW], pj)
                else:
                    for sc in range(NSC):
                        pv = ps.tile([128, E_LOCAL], F32, tag="work", bufs=2,
                                     name=f"pv_{st}_{sc}")
                        for dc in range(NDC):
                            nc.tensor.matmul(
                                pv, xt[:, dc, sc * 128:(sc + 1) * 128],
                                wv_sb[:, dc, :],
                                start=(dc == 0), stop=(dc == NDC - 1))
                        ci = st * NSC + sc
                        nc.vector.tensor_copy(
                            v_sb[ci][:, :, 0:64],
                            pv.rearrange("p (h e) -> p h e", h=H_LOCAL))
                        ones_b = bass.AP(
                            tensor=ones_col.tensor, offset=ones_col.offset,
                            ap=[ones_col.ap[0], [0, H_LOCAL],
                                ones_col.ap[1]],
                        )
                        nc.vector.tensor_copy(v_sb[ci][:, :, 64:65], ones_b)

            def attention_unit(j, hp):
                q0 = j * 512
                nlast = 4 * j + 3
                qt = qt_rot[(j, hp)]
                cpx = [ps.tile([65, 512], F32, tag="cpx", bufs=2,
                               name=f"cpx{hp}_{j}_{hi}") for hi in range(2)]
                ctx_rot[(j, hp)] = sm.tile([128, 512], F32R, tag=f"ctxr{hp}",
                                           bufs=2, name=f"ctxr{hp}_{j}")
                for c in range(4 * j + 4):
                    vo = max(0, c * 128 - q0)
                    lg = ps.tile([128, 1024], F32, tag="lg", bufs=2,
                                 name=f"lg{hp}_{j}_{c}")
                    pt = sm.tile([128, 1024], F32R, tag="pt", bufs=2,
                                 name=f"pt{hp}_{j}_{c}")
                    for hi in range(2):
                        nc.tensor.matmul(
                            lg[:, hi * 512 + vo:(hi + 1) * 512],
                            kt_sb[hp][hi * 64:(hi + 1) * 64,
                                      c * 128:(c + 1) * 128],
                            qt[hi * 64:(hi + 1) * 64, vo:512],
                            start=True, stop=True)
                    if c >= 4 * j:
                        m = c - 4 * j
                        blk = lg.rearrange("p (hh q) -> p hh q", hh=2)[
                            :, :, m * 128:(m + 1) * 128]
                        tri_b = bass.AP(
                            tensor=trimask.tensor, offset=trimask.offset,
                            ap=[trimask.ap[0], [0, 2], trimask.ap[1]],
                        )
                        nc.vector.tensor_add(blk, blk, tri_b)
                    nc.scalar.activation(pt[:, vo:1024], lg[:, vo:1024], Exp)
                    for hi in range(2):
                        nc.tensor.matmul(
                            cpx[hi][:, vo:512],
                            v_sb[c][:, hp * 2 + hi, :],
                            pt[:, hi * 512 + vo:(hi + 1) * 512],
                            start=(c == 0), stop=(c == nlast))
                for hi in range(2):
                    bcast = sm.tile([64, 512], F32, tag="bcast", bufs=1,
                                    name=f"bc{hp}_{j}_{hi}")
                    nc.vector.tensor_copy(bcast[0:1, :], cpx[hi][64:65, :])
                    nc.vector.reciprocal_approx_fast(
                        out=bcast[0:1, :], in_=bcast[0:1, :])
                    nc.gpsimd.partition_broadcast(bcast, bcast[0:1, :])
                    nc.vector.tensor_mul(
                        ctx_rot[(j, hp)][hi * 64:(hi + 1) * 64, :],
                        cpx[hi][0:64, :], bcast)

            for st in (0, 1):
                for t in ("q", "k", "v"):
                    proj_unit(st, t)

            # Emission = program order: every producer must be emitted
            # before its consumers.  Projection for window j+1 is emitted
            # interleaved with attention(j) as PE filler; attention(3)
            # (the largest window, no projection left) gets the deferred
            # out-projection of window 2 as filler instead.
            fills = {
                0: [(2, "q"), (2, "k"), (2, "v"), (3, "q"), (3, "k"),
                    (3, "v")],
                1: [(4, "q"), (4, "k"), (4, "v"), (5, "q"), (5, "k"),
                    (5, "v")],
                2: [(6, "q"), (6, "k"), (6, "v"), (7, "q"), (7, "k"),
                    (7, "v")],
                3: [],
            }

            def out_block(qc):
                for nh in range(2):
                    po = ps.tile([128, 512], F32, tag="work", bufs=2,
                                 name=f"po{qc}_{nh}")
                    for dvc in range(NEC):
                        nc.tensor.matmul(
                            po,
                            ctx_rot[(qc // 4, dvc)][:,
                                                    (qc % 4) * 128:
                                                    (qc % 4 + 1) * 128],
                            wo_sb[:, dvc, nh * 512:(nh + 1) * 512],
                            start=(dvc == 0), stop=(dvc == NEC - 1))
                    osb = sm.tile([128, 512], F32, tag="osb", bufs=1,
                                  name=f"osb{qc}_{nh}")
                    nc.scalar.copy(osb, po)
                    nc.sync.dma_start(
                        out=out[qc * 128:(qc + 1) * 128,
                                nh * 512:(nh + 1) * 512],
                        in_=osb)

            for j in range(3):
                fill = list(fills[j])
                for hp in range(NEC):
                    attention_unit(j, hp)
                    for _ in range(2):
                        if fill:
                            proj_unit(*fill.pop(0))
                while fill:
                    proj_unit(*fill.pop(0))
                if j < 2:
                    for qc in range(4 * j, 4 * j + 4):
                        out_block(qc)
            # j = 3: out(2) blocks act as the PE filler
            for hp in range(NEC):
                attention_unit(3, hp)
                out_block(8 + hp)
            for qc in range(12, 16):
                out_block(qc)

    nc.compile()
    return nc


def kernel(queries, keys, values, mask=None, Wq=None, Wk=None, Wv=None,
           Wo=None, **_ignored):
    from concourse.bass_utils import run_bass_kernel_spmd

    if "nc" not in _cached:
        _cached["nc"] = _build()
    nc = _cached["nc"]

    scale = np.float32(0.125)  # (DK//H) ** -0.5, exact power of two
    in_maps = []
    for c in range(N_CORES):
        b, g = divmod(c, 2)
        sl = slice(g * E_LOCAL, (g + 1) * E_LOCAL)
        in_maps.append({
            "xq": np.ascontiguousarray(queries[b], dtype=np.float32),
            "xk": np.ascontiguousarray(keys[b], dtype=np.float32),
            "xv": np.ascontiguousarray(values[b], dtype=np.float32),
            "wq": np.ascontiguousarray(Wq[:, sl] * scale),
            "wk": np.ascontiguousarray(Wk[:, sl]),
            "wv": np.ascontiguousarray(Wv[:, sl]),
            "wo": np.ascontiguousarray(Wo[sl, :]),
        })
    res = run_bass_kernel_spmd(nc, in_maps, core_ids=list(range(N_CORES)))
    outs = res.results
    full = np.empty((B, S, HID), np.float32)
    for b in range(B):
        full[b] = outs[2 * b]["out"] + outs[2 * b + 1]["out"]
    return full



# revision 2
# speedup vs baseline: 16648.6591x; 16648.6591x over previous
"""Multi-head causal attention kernel for Trainium2 (8 NeuronCores).

Problem: B=4, S=2048, HID=1024, H=16 heads (head_dim 64), causal mask,
fp32 I/O.  out = softmax(mask + (XqWq)(XkWk)^T/8) (XvWv) Wo

Sharding: 8 cores = 4 batches x 2 head-groups.  Core c handles batch
c//2 and heads (c%2)*8 .. +8 (dk slice of 512).  Each core computes a
full-shape [S, HID] partial output (its head-group's contribution
through Wo); the host sums the two partials per batch.

Per-core dataflow (all matmuls in float32r = TF32-like, full PE rate):
  - PE-transpose X chunks -> X^T; project to kT (persistent, [e,s]
    layout, 2 heads per 128-partition tile), qT (rotating per-512-q
    window) and v (persistent, natural [s,e] with a ones column per
    head so the PV matmul also emits softmax denominators).
  - Attention in transposed [k,q] orientation per (q-window j, head
    pair): logits^T = kT-chunk (stationary) x qT (moving) with
    causally-restricted columns; additive triangular mask on diagonal
    blocks (DVE, in PSUM); exp on ScalarE PSUM->SBUF; PV accumulates
    ctx^T in PSUM (per-element has_written makes partial-range
    accumulation correct).  Denominator row -> reciprocal (DVE) ->
    partition_broadcast (GpSimd) -> multiply-evacuate ctx^T (DVE).
  - Output projection ctx^T.T @ Wo per q-window, fused into the stream.

The projection work for q-window j+1 is emitted interleaved with the
attention work of window j: the projection matmuls act as PE filler
that keeps the PE HAM activity monitor busy (otherwise the exp-bound
attention inner loop lets the PE clock-gate down to 1.2 GHz).
"""

import numpy as np

B, S, HID = 4, 2048, 1024
H_LOCAL, E_LOCAL = 8, 512  # heads / dk columns handled per core
N_CORES = 8
USE_F32R = True

_cached = {}


def _build():
    from concourse import bacc
    import concourse.bass as bass
    import concourse.mybir as mybir
    import concourse.tile as tile
    from concourse.masks import make_identity

    F32 = mybir.dt.float32
    F32R = mybir.dt.float32r if USE_F32R else mybir.dt.float32
    Exp = mybir.ActivationFunctionType.Exp

    nc = bacc.Bacc()
    xq = nc.dram_tensor("xq", [S, HID], F32R, kind="ExternalInput")
    xk = nc.dram_tensor("xk", [S, HID], F32R, kind="ExternalInput")
    xv = nc.dram_tensor("xv", [S, HID], F32R, kind="ExternalInput")
    wq = nc.dram_tensor("wq", [HID, E_LOCAL], F32R, kind="ExternalInput")
    wk = nc.dram_tensor("wk", [HID, E_LOCAL], F32R, kind="ExternalInput")
    wv = nc.dram_tensor("wv", [HID, E_LOCAL], F32R, kind="ExternalInput")
    wo = nc.dram_tensor("wo", [E_LOCAL, HID], F32R, kind="ExternalInput")
    out = nc.dram_tensor("out", [S, HID], F32, kind="ExternalOutput")

    NST = 8           # projection s-tiles
    STW = S // NST    # 256 rows per s-tile
    NSC = STW // 128  # 2 s-chunks per s-tile
    NDC = HID // 128  # 8 d-chunks
    NEC = E_LOCAL // 128  # 4 e-chunks = head pairs
    NKC = S // 128    # 16 k-chunks
    NQT = 4           # q windows of 512

    with tile.TileContext(nc) as tc:
        with (
            tc.sbuf_pool(name="consts", bufs=1) as consts,
            tc.sbuf_pool(name="persist", bufs=1) as persist,
            tc.sbuf_pool(name="stream", bufs=1) as sm,
            tc.psum_pool(name="ps", bufs=1) as ps,
        ):
            ident_f = consts.tile([128, 128], F32)
            make_identity(nc, ident_f)
            ident = consts.tile([128, 128], F32R)
            nc.vector.tensor_copy(ident, ident_f)
            # additive causal mask for diagonal [k,q] blocks: 0 where
            # k <= q else -1e9
            trimask = consts.tile([128, 128], F32)
            nc.gpsimd.memset(trimask, 0.0)
            nc.gpsimd.affine_select(
                out=trimask, in_=trimask,
                compare_op=mybir.AluOpType.is_ge, fill=-1e9, base=0,
                pattern=[[1, 128]], channel_multiplier=-1,
            )
            ones_col = consts.tile([128, 1], F32)
            nc.vector.memset(ones_col, 1.0)

            kt_sb = [persist.tile([128, S], F32R, name=f"kt{i}", tag=f"kt{i}")
                     for i in range(NEC)]
            v_sb = [persist.tile([128, H_LOCAL, 65], F32R, name=f"v{i}",
                                 tag=f"v{i}") for i in range(NKC)]

            wq_sb = sm.tile([128, NDC, E_LOCAL], F32R, tag="wq", bufs=1)
            wk_sb = sm.tile([128, NDC, E_LOCAL], F32R, tag="wk", bufs=1)
            wv_sb = sm.tile([128, NDC, E_LOCAL], F32R, tag="wv", bufs=1)
            wo_sb = sm.tile([128, NEC, HID], F32R, tag="wo", bufs=1)
            nc.sync.dma_start(
                out=wq_sb, in_=wq.rearrange("(dc p) e -> p dc e", p=128))
            nc.sync.dma_start(
                out=wk_sb, in_=wk.rearrange("(dc p) e -> p dc e", p=128))
            nc.sync.dma_start(
                out=wv_sb, in_=wv.rearrange("(dc p) e -> p dc e", p=128))
            nc.sync.dma_start(
                out=wo_sb, in_=wo.rearrange("(dv p) n -> p dv n", p=128))

            qt_rot = {}   # (window, ec) -> [128, 512] tile
            ctx_rot = {}  # (window, hp) -> [128, 512] tile

            def proj_unit(st, tname):
                """Load + transpose + project one input tensor s-tile."""
                s0 = st * STW
                w = st // 2
                xdram = {"q": xq, "k": xk, "v": xv}[tname]
                xnat = sm.tile([128, NSC, HID], F32R, tag="xnat", bufs=2,
                               name=f"xnat_{tname}{st}")
                nc.sync.dma_start(
                    out=xnat,
                    in_=xdram[s0:s0 + STW, :].rearrange(
                        "(sc p) d -> p sc d", p=128))
                xt = sm.tile([128, NDC, STW], F32R, tag="xt", bufs=2,
                             name=f"xt_{tname}{st}")
                for dcp in range(NDC // 2):
                    tp = ps.tile([128, 512], F32R, tag="work", bufs=2,
                                 name=f"tp_{tname}{st}_{dcp}")
                    for k2 in range(2):
                        dc = dcp * 2 + k2
                        for sc in range(NSC):
                            nc.tensor.transpose(
                                tp[:, k2 * STW + sc * 128:
                                   k2 * STW + (sc + 1) * 128],
                                xnat[:, sc, dc * 128:(dc + 1) * 128],
                                ident)
                    nc.vector.tensor_copy(xt[:, dcp * 2:dcp * 2 + 2, :], tp)

                if tname == "q":
                    for ec in range(NEC):
                        if st % 2 == 0:
                            qt_rot[(w, ec)] = sm.tile(
                                [128, 512], F32R, tag=f"qtr{ec}", bufs=2,
                                name=f"qtr{ec}_{w}")
                        pj = ps.tile([128, STW], F32, tag="work", bufs=2,
                                     name=f"pjq_{st}_{ec}")
                        for dc in range(NDC):
                            nc.tensor.matmul(
                                pj, wq_sb[:, dc, ec * 128:(ec + 1) * 128],
                                xt[:, dc, :],
                                start=(dc == 0), stop=(dc == NDC - 1))
                        off = (st % 2) * STW
                        nc.vector.tensor_copy(
                            qt_rot[(w, ec)][:, off:off + STW], pj)
                elif tname == "k":
                    for ec in range(NEC):
                        pj = ps.tile([128, STW], F32, tag="work", bufs=2,
                                     name=f"pjk_{st}_{ec}")
                        for dc in range(NDC):
                            nc.tensor.matmul(
                                pj, wk_sb[:, dc, ec * 128:(ec + 1) * 128],
                                xt[:, dc, :],
                                start=(dc == 0), stop=(dc == NDC - 1))
                        nc.vector.tensor_copy(
                            kt_sb[ec][:, s0:s0 + STW], pj)
                else:
                    for sc in range(NSC):
                        pv = ps.tile([128, E_LOCAL], F32, tag="work", bufs=2,
                                     name=f"pv_{st}_{sc}")
                        for dc in range(NDC):
                            nc.tensor.matmul(
                                pv, xt[:, dc, sc * 128:(sc + 1) * 128],
                                wv_sb[:, dc, :],
                                start=(dc == 0), stop=(dc == NDC - 1))
                        ci = st * NSC + sc
                        nc.vector.tensor_copy(
                            v_sb[ci][:, :, 0:64],
                            pv.rearrange("p (h e) -> p h e", h=H_LOCAL))
                        ones_b = bass.AP(
                            tensor=ones_col.tensor, offset=ones_col.offset,
                            ap=[ones_col.ap[0], [0, H_LOCAL],
                                ones_col.ap[1]],
                        )
                        nc.vector.tensor_copy(v_sb[ci][:, :, 64:65], ones_b)

            def attention_unit(j, hp):
                q0 = j * 512
                nlast = 4 * j + 3
                qt = qt_rot[(j, hp)]
                cpx = [ps.tile([65, 512], F32, tag="cpx", bufs=2,
                               name=f"cpx{hp}_{j}_{hi}") for hi in range(2)]
                ctx_rot[(j, hp)] = sm.tile([128, 512], F32R, tag=f"ctxr{hp}",
                                           bufs=2, name=f"ctxr{hp}_{j}")
                for c in range(4 * j + 4):
                    vo = max(0, c * 128 - q0)
                    lg = ps.tile([128, 1024], F32, tag="lg", bufs=2,
                                 name=f"lg{hp}_{j}_{c}")
                    pt = sm.tile([128, 1024], F32R, tag="pt", bufs=2,
                                 name=f"pt{hp}_{j}_{c}")
                    for hi in range(2):
                        nc.tensor.matmul(
                            lg[:, hi * 512 + vo:(hi + 1) * 512],
                            kt_sb[hp][hi * 64:(hi + 1) * 64,
                                      c * 128:(c + 1) * 128],
                            qt[hi * 64:(hi + 1) * 64, vo:512],
                            start=True, stop=True)
                    if c >= 4 * j:
                        m = c - 4 * j
                        blk = lg.rearrange("p (hh q) -> p hh q", hh=2)[
                            :, :, m * 128:(m + 1) * 128]
                        tri_b = bass.AP(
                            tensor=trimask.tensor, offset=trimask.offset,
                            ap=[trimask.ap[0], [0, 2], trimask.ap[1]],
                        )
                        nc.vector.tensor_add(blk, blk, tri_b)
                    nc.scalar.activation(pt[:, vo:1024], lg[:, vo:1024], Exp)
                    for hi in range(2):
                        nc.tensor.matmul(
                            cpx[hi][:, vo:512],
                            v_sb[c][:, hp * 2 + hi, :],
                            pt[:, hi * 512 + vo:(hi + 1) * 512],
                            start=(c == 0), stop=(c == nlast))
                for hi in range(2):
                    bcast = sm.tile([64, 512], F32, tag="bcast", bufs=1,
                                    name=f"bc{hp}_{j}_{hi}")
                    nc.vector.tensor_copy(bcast[0:1, :], cpx[hi][64:65, :])
                    nc.vector.reciprocal_approx_fast(
                        out=bcast[0:1, :], in_=bcast[0:1, :])
                    nc.gpsimd.partition_broadcast(bcast, bcast[0:1, :])
                    nc.vector.tensor_mul(
                        ctx_rot[(j, hp)][hi * 64:(hi + 1) * 64, :],
                        cpx[hi][0:64, :], bcast)

            for st in (0, 1):
                for t in ("q", "k", "v"):
                    proj_unit(st, t)

            # Emission = program order: every producer must be emitted
            # before its consumers.  Projection for window j+1 is emitted
            # interleaved with attention(j) as PE filler; attention(3)
            # (the largest window, no projection left) gets the deferred
            # out-projection of window 2 as filler instead.
            fills = {
                0: [(2, "q"), (2, "k"), (2, "v"), (3, "q"), (3, "k"),
                    (3, "v")],
                1: [(4, "q"), (4, "k"), (4, "v"), (5, "q"), (5, "k"),
                    (5, "v")],
                2: [(6, "q"), (6, "k"), (6, "v"), (7, "q"), (7, "k"),
                    (7, "v")],
                3: [],
            }

            def out_block(qc):
                for nh in range(2):
                    po = ps.tile([128, 512], F32, tag="work", bufs=2,
                                 name=f"po{qc}_{nh}")
                    for dvc in range(NEC):
                        nc.tensor.matmul(
                            po,
                            ctx_rot[(qc // 4, dvc)][:,
                                                    (qc % 4) * 128:
                                                    (qc % 4 + 1) * 128],
                            wo_sb[:, dvc, nh * 512:(nh + 1) * 512],
                            start=(dvc == 0), stop=(dvc == NEC - 1))
                    osb = sm.tile([128, 512], F32, tag="osb", bufs=1,
                                  name=f"osb{qc}_{nh}")
                    nc.scalar.copy(osb, po)
                    nc.sync.dma_start(
                        out=out[qc * 128:(qc + 1) * 128,
                                nh * 512:(nh + 1) * 512],
                        in_=osb)

            for j in range(3):
                fill = list(fills[j])
                for hp in range(NEC):
                    attention_unit(j, hp)
                    for _ in range(2):
                        if fill:
                            proj_unit(*fill.pop(0))
                while fill:
                    proj_unit(*fill.pop(0))
                if j < 2:
                    for qc in range(4 * j, 4 * j + 4):
                        out_block(qc)
            # j = 3: out(2) blocks act as the PE filler
            for hp in range(NEC):
                attention_unit(3, hp)
                out_block(8 + hp)
            for qc in range(12, 16):
                out_block(qc)

    nc.compile()
    return nc


def _in_maps(queries, keys, values, Wq, Wk, Wv, Wo):
    scale = np.float32(0.125)  # (DK//H) ** -0.5, exact power of two
    in_maps = []
    for c in range(N_CORES):
        b, g = divmod(c, 2)
        sl = slice(g * E_LOCAL, (g + 1) * E_LOCAL)
        in_maps.append({
            "xq": np.ascontiguousarray(queries[b], dtype=np.float32),
            "xk": np.ascontiguousarray(keys[b], dtype=np.float32),
            "xv": np.ascontiguousarray(values[b], dtype=np.float32),
            "wq": np.ascontiguousarray(Wq[:, sl] * scale),
            "wk": np.ascontiguousarray(Wk[:, sl]),
            "wv": np.ascontiguousarray(Wv[:, sl]),
            "wo": np.ascontiguousarray(Wo[sl, :]),
        })
    return in_maps


def kernel(queries, keys, values, mask=None, Wq=None, Wk=None, Wv=None,
           Wo=None, **_ignored):
    from concourse.bass_utils import run_bass_kernel_spmd

    if "nc" not in _cached:
        _cached["nc"] = _build()
    nc = _cached["nc"]

    in_maps = _in_maps(queries, keys, values, Wq, Wk, Wv, Wo)
    res = run_bass_kernel_spmd(nc, in_maps, core_ids=list(range(N_CORES)))
    outs = res.results
    full = np.empty((B, S, HID), np.float32)
    for b in range(B):
        full[b] = outs[2 * b]["out"] + outs[2 * b + 1]["out"]
    return full


def run_traced(inputs, tmpdir=None):
    """Run once with NTFF tracing; returns BassKernelResults."""
    from concourse.bass_utils import run_bass_kernel_spmd

    if "nc" not in _cached:
        _cached["nc"] = _build()
    nc = _cached["nc"]
    in_maps = _in_maps(inputs["queries"], inputs["keys"], inputs["values"],
                       inputs["Wq"], inputs["Wk"], inputs["Wv"], inputs["Wo"])
    return run_bass_kernel_spmd(nc, in_maps, core_ids=list(range(N_CORES)),
                                trace=True, tmpdir=tmpdir)



# revision 14
# speedup vs baseline: 27232.7873x; 1.6357x over previous
"""Multi-head causal attention kernel for Trainium2 (8 NeuronCores).

Problem: B=4, S=2048, HID=1024, H=16 heads (head_dim 64), causal mask,
fp32 I/O.  out = softmax(mask + (XqWq)(XkWk)^T/8) (XvWv) Wo

Sharding: 8 cores = 4 batches x 2 head-groups.  Core c handles batch
c//2 and heads (c%2)*8 .. +8 (dk slice of 512).  Each core computes a
full-shape [S, HID] partial output (its head-group's contribution
through Wo); the host sums the two partials per batch.

v2 design (vs the 495us baseline):
  - X is transposed to [d, s] and cast to bf16 on the HOST, so the
    kernel needs no PE-transposes and no PSUM->SBUF transpose
    evacuations (the baseline burned ~107us of DVE and ~20us of PE on
    those).  All matmul operands are bf16 (same PE rate, half the DMA
    and SBUF traffic, 2x DVE modes on evacuations).
  - Attention runs in transposed [k, q] orientation per 512-wide
    q-window and head-pair: logits^T = kT-chunk x qT, exp on ScalarE
    (PSUM->SBUF, bf16 out), causal masking done by ZEROING the exp'd
    upper-triangle of the diagonal blocks on the otherwise-idle GpSimd
    engine (affine_select), PV accumulates ctx^T in PSUM with a ones
    column in v producing softmax denominators for free.
  - Normalization: reciprocal straight off the PSUM denominator row,
    GpSimd partition-broadcast, one DVE multiply-evacuate.
  - Projection/out-projection matmuls are emitted in ~4-MM "filler"
    items interleaved between attention chunks so the PE stays dense
    (HAM stays at full clock) while ScalarE grinds exps.
"""

import numpy as np

B, S, HID = 4, 2048, 1024
H_LOCAL, E_LOCAL = 8, 512  # heads / dk columns handled per core
N_CORES = 8

_cached = {}
DEBUG = False


def _build():
    from concourse import bacc
    import concourse.bass as bass
    import concourse.mybir as mybir
    import concourse.tile as tile

    F32 = mybir.dt.float32
    BF16 = mybir.dt.bfloat16
    Exp = mybir.ActivationFunctionType.Exp

    NDC = HID // 128   # 8 d-chunks
    NEC = E_LOCAL // 128  # 4 e-chunks = head pairs
    NKC = S // 128     # 16 k-chunks
    W = 512            # q-window
    NW = S // W        # 4 windows

    nc = bacc.Bacc()
    xqt = nc.dram_tensor("xqt", [HID, S], BF16, kind="ExternalInput")
    xkt = nc.dram_tensor("xkt", [HID, S], BF16, kind="ExternalInput")
    xvt = nc.dram_tensor("xvt", [HID, S], BF16, kind="ExternalInput")
    wq = nc.dram_tensor("wq", [HID, E_LOCAL], BF16, kind="ExternalInput")
    wk = nc.dram_tensor("wk", [HID, E_LOCAL], BF16, kind="ExternalInput")
    wv = nc.dram_tensor("wv", [HID, E_LOCAL], BF16, kind="ExternalInput")
    wo = nc.dram_tensor("wo", [E_LOCAL, HID], BF16, kind="ExternalInput")
    out = nc.dram_tensor("out", [S, HID], F32, kind="ExternalOutput")
    if DEBUG:
        dbg_qt = nc.dram_tensor("dbg_qt", [128, 512], F32,
                                kind="ExternalOutput")
        dbg_kt = nc.dram_tensor("dbg_kt", [128, 512], F32,
                                kind="ExternalOutput")
        dbg_v = nc.dram_tensor("dbg_v", [128, 8, 65], F32,
                               kind="ExternalOutput")
        dbg_den = nc.dram_tensor("dbg_den", [2, 512], F32,
                                 kind="ExternalOutput")
        dbg_cpx = nc.dram_tensor("dbg_cpx", [64, 512], F32,
                                 kind="ExternalOutput")
        dbg_ctx = nc.dram_tensor("dbg_ctx", [128, 512], F32,
                                 kind="ExternalOutput")
        dbg_pt = nc.dram_tensor("dbg_pt", [128, 1024], F32,
                                kind="ExternalOutput")
        dbg_bc = nc.dram_tensor("dbg_bc", [2, 64, 512], F32,
                                kind="ExternalOutput")

    with tile.TileContext(nc) as tc:
        with (
            tc.sbuf_pool(name="consts", bufs=1) as consts,
            tc.sbuf_pool(name="persist", bufs=1) as persist,
            tc.sbuf_pool(name="sm", bufs=1) as sm,
            tc.psum_pool(name="ps", bufs=1) as ps,
        ):
            ones_col = consts.tile([128, 1], BF16)
            nc.vector.memset(ones_col, 1.0)

            wq_sb = persist.tile([128, NDC, E_LOCAL], BF16, tag="wq")
            wk_sb = persist.tile([128, NDC, E_LOCAL], BF16, tag="wk")
            wv_sb = persist.tile([128, NDC, E_LOCAL], BF16, tag="wv")
            wo_sb = persist.tile([128, NEC, HID], BF16, tag="wo")

            kt_sb = [persist.tile([128, S], BF16, tag=f"kt{i}", name=f"kt{i}")
                     for i in range(NEC)]
            v_sb = [persist.tile([128, H_LOCAL, 65], BF16, tag=f"v{i}", name=f"v{i}")
                    for i in range(NKC)]
            qt = {}   # (w, ec) -> [128, W] bf16
            ctx = {}  # (w, hp) -> [128, W] bf16

            xdram = {"q": xqt, "k": xkt, "v": xvt}
            xt_tiles = {}

            def load_x(tname, w):
                t = sm.tile([128, NDC, W], BF16, tag=f"x{tname}", bufs=2,
                            name=f"x{tname}_{w}")
                nc.sync.dma_start(
                    out=t,
                    in_=xdram[tname][:, w * W:(w + 1) * W].rearrange(
                        "(dc p) s -> p dc s", p=128))
                xt_tiles[(tname, w)] = t

            # weights + first two x-windows; ordering puts the tensors
            # needed first at the head of the DMA queues.
            nc.sync.dma_start(
                out=wq_sb, in_=wq.rearrange("(dc p) e -> p dc e", p=128))
            load_x("q", 0)
            nc.sync.dma_start(
                out=wk_sb, in_=wk.rearrange("(dc p) e -> p dc e", p=128))
            load_x("k", 0)
            nc.sync.dma_start(
                out=wv_sb, in_=wv.rearrange("(dc p) e -> p dc e", p=128))
            load_x("v", 0)
            nc.sync.dma_start(
                out=wo_sb, in_=wo.rearrange("(dv p) n -> p dv n", p=128))
            load_x("q", 1)
            load_x("k", 1)
            load_x("v", 1)

            def proj_items(w):
                """Emission closures (~4 matmuls each) projecting window w."""
                items = []
                if w >= 2:
                    def dma_item(w=w):
                        load_x("q", w)
                        load_x("k", w)
                        load_x("v", w)
                    items.append(dma_item)

                holder = {}

                def qk_first(tname, ec, w=w):
                    wsb = wq_sb if tname == "q" else wk_sb
                    pj = ps.tile([128, W], F32, tag="work", bufs=2,
                                 name=f"pj{tname}{w}_{ec}")
                    holder[(tname, ec)] = pj
                    xt = xt_tiles[(tname, w)]
                    for dc in range(4):
                        nc.tensor.matmul(
                            pj, wsb[:, dc, ec * 128:(ec + 1) * 128],
                            xt[:, dc, :], start=(dc == 0), stop=False)

                def qk_second(tname, ec, w=w):
                    wsb = wq_sb if tname == "q" else wk_sb
                    pj = holder.pop((tname, ec))
                    xt = xt_tiles[(tname, w)]
                    for dc in range(4, NDC):
                        nc.tensor.matmul(
                            pj, wsb[:, dc, ec * 128:(ec + 1) * 128],
                            xt[:, dc, :], start=False, stop=(dc == NDC - 1))
                    if tname == "q":
                        qt[(w, ec)] = persist.tile(
                            [128, W], BF16, tag=f"qt{w}_{ec}", name=f"qt{w}_{ec}")
                        nc.vector.tensor_copy(qt[(w, ec)], pj)
                    else:
                        nc.vector.tensor_copy(
                            kt_sb[ec][:, w * W:(w + 1) * W], pj)

                def v_first(sc, w=w):
                    pv = ps.tile([128, E_LOCAL], F32, tag="work", bufs=2,
                                 name=f"pv{w}_{sc}")
                    holder[("v", sc)] = pv
                    xt = xt_tiles[("v", w)]
                    for dc in range(4):
                        nc.tensor.matmul(
                            pv, xt[:, dc, sc * 128:(sc + 1) * 128],
                            wv_sb[:, dc, :], start=(dc == 0), stop=False)

                def v_second(sc, w=w):
                    pv = holder.pop(("v", sc))
                    xt = xt_tiles[("v", w)]
                    for dc in range(4, NDC):
                        nc.tensor.matmul(
                            pv, xt[:, dc, sc * 128:(sc + 1) * 128],
                            wv_sb[:, dc, :], start=False, stop=(dc == NDC - 1))
                    ci = w * 4 + sc
                    nc.vector.tensor_copy(
                        v_sb[ci][:, :, 0:64],
                        pv.rearrange("p (h e) -> p h e", h=H_LOCAL))
                    ones_b = bass.AP(
                        tensor=ones_col.tensor, offset=ones_col.offset,
                        ap=[ones_col.ap[0], [0, H_LOCAL], ones_col.ap[1]])
                    nc.vector.tensor_copy(v_sb[ci][:, :, 64:65], ones_b)

                for ec in range(NEC):
                    items.append(lambda ec=ec: qk_first("q", ec))
                    items.append(lambda ec=ec: qk_second("q", ec))
                for ec in range(NEC):
                    items.append(lambda ec=ec: qk_first("k", ec))
                    items.append(lambda ec=ec: qk_second("k", ec))
                for sc in range(4):
                    items.append(lambda sc=sc: v_first(sc))
                    items.append(lambda sc=sc: v_second(sc))
                return items

            def out_items(w):
                """Out-projection of window w (needs ctx[(w, *)])."""
                items = []

                def emit(qc, nh):
                    po = ps.tile([128, 512], F32, tag="work", bufs=2,
                                 name=f"po{qc}_{nh}")
                    for dvc in range(NEC):
                        nc.tensor.matmul(
                            po,
                            ctx[(w, dvc)][:, (qc % 4) * 128:
                                          (qc % 4 + 1) * 128],
                            wo_sb[:, dvc, nh * 512:(nh + 1) * 512],
                            start=(dvc == 0), stop=(dvc == NEC - 1))
                    osb = sm.tile([128, 512], F32, tag="osb", bufs=2,
                                  name=f"osb{qc}_{nh}")
                    nc.vector.tensor_copy(osb, po)
                    nc.sync.dma_start(
                        out=out[qc * 128:(qc + 1) * 128,
                                nh * 512:(nh + 1) * 512],
                        in_=osb)

                for qc in range(4 * w, 4 * w + 4):
                    for nh in range(2):
                        items.append(lambda qc=qc, nh=nh: emit(qc, nh))
                return items

            def attention_unit(j, hp, tick):
                q0 = j * W
                nlast = 4 * j + 3
                qtile = qt[(j, hp)]
                cpx = [ps.tile([65, W], F32, tag="cpx", bufs=2,
                               name=f"cpx{j}_{hp}_{hi}") for hi in range(2)]
                ctx[(j, hp)] = persist.tile([128, W], BF16, tag=f"ctx{j}_{hp}",
                                            name=f"ctx{j}_{hp}")
                for c in range(4 * j + 4):
                    vo = max(0, c * 128 - q0)
                    lg = ps.tile([128, 2 * W], F32, tag="lg", bufs=2,
                                 name=f"lg{j}_{hp}_{c}")
                    pt = sm.tile([128, 2 * W], BF16, tag="pt", bufs=3,
                                 name=f"pt{j}_{hp}_{c}")
                    for hi in range(2):
                        nc.tensor.matmul(
                            lg[:, hi * W + vo:(hi + 1) * W],
                            kt_sb[hp][hi * 64:(hi + 1) * 64,
                                      c * 128:(c + 1) * 128],
                            qtile[hi * 64:(hi + 1) * 64, vo:W],
                            start=True, stop=True)
                    nc.scalar.activation(pt[:, vo:2 * W], lg[:, vo:2 * W],
                                         Exp)
                    if c >= 4 * j:
                        # zero the exp'd upper triangle of the diagonal
                        # 128-block of each head (replaces the -1e9 mask)
                        blk = pt.rearrange("p (h q) -> p h q", h=2)[
                            :, :, vo:vo + 128]
                        nc.gpsimd.affine_select(
                            out=blk, in_=blk,
                            compare_op=mybir.AluOpType.is_ge, fill=0.0,
                            base=0, pattern=[[0, 2], [1, 128]],
                            channel_multiplier=-1)
                    if DEBUG and j == 0 and hp == 0 and c == 0:
                        dbg_pt_sb = sm.tile([128, 1024], F32, tag="dbgpt",
                                            bufs=1)
                        nc.vector.tensor_copy(dbg_pt_sb, pt)
                        nc.sync.dma_start(out=dbg_pt[:, :], in_=dbg_pt_sb)
                    for hi in range(2):
                        nc.tensor.matmul(
                            cpx[hi][:, vo:W],
                            v_sb[c][:, hp * 2 + hi, :],
                            pt[:, hi * W + vo:(hi + 1) * W],
                            start=(c == 0), stop=(c == nlast))
                    tick()
                if DEBUG and j == 0 and hp == 0:
                    for hi in range(2):
                        dsb = sm.tile([1, 512], F32, tag=f"dbgden{hi}",
                                      bufs=1, name=f"dbgden{hi}")
                        nc.vector.tensor_copy(dsb, cpx[hi][64:65, :])
                        nc.sync.dma_start(out=dbg_den[hi:hi + 1, :], in_=dsb)
                    csb = sm.tile([64, 512], F32, tag="dbgcpx", bufs=1)
                    nc.vector.tensor_copy(csb, cpx[0][0:64, :])
                    nc.sync.dma_start(out=dbg_cpx[:, :], in_=csb)
                for hi in range(2):
                    bc = sm.tile([64, W], F32, tag="bc", bufs=2,
                                 name=f"bc{j}_{hp}_{hi}")
                    # NB: reciprocal_approx_fast is a custom-DVE op that
                    # drops the input AP's base partition, so the PSUM
                    # denominator row must be copied to partition 0 first.
                    nc.vector.tensor_copy(bc[0:1, :], cpx[hi][64:65, :])
                    nc.vector.reciprocal_approx_fast(
                        out=bc[0:1, :], in_=bc[0:1, :])
                    nc.gpsimd.partition_broadcast(bc, bc[0:1, :])
                    if DEBUG and j == 0 and hp == 0:
                        bsb = sm.tile([64, 512], F32, tag=f"dbgbc{hi}",
                                      bufs=1, name=f"dbgbc{hi}")
                        nc.vector.tensor_copy(bsb, bc)
                        nc.sync.dma_start(out=dbg_bc[hi, :, :], in_=bsb)
                    nc.vector.tensor_mul(
                        ctx[(j, hp)][hi * 64:(hi + 1) * 64, :],
                        cpx[hi][0:64, :], bc)

            # ---- schedule ----
            for it in proj_items(0):
                it()
            if DEBUG:
                t1 = sm.tile([128, 512], F32, tag="dbg1", bufs=1)
                nc.vector.tensor_copy(t1, qt[(0, 0)])
                nc.sync.dma_start(out=dbg_qt[:, :], in_=t1)
                t2 = sm.tile([128, 512], F32, tag="dbg2", bufs=1)
                nc.vector.tensor_copy(t2, kt_sb[0][:, 0:512])
                nc.sync.dma_start(out=dbg_kt[:, :], in_=t2)
                t3 = sm.tile([128, 8, 65], F32, tag="dbg3", bufs=1)
                nc.vector.tensor_copy(t3, v_sb[0])
                nc.sync.dma_start(out=dbg_v[:, :, :], in_=t3)

            phase_fill = {
                0: proj_items(1),
                1: proj_items(2),
                2: proj_items(3),
                3: out_items(0) + out_items(1) + out_items(2),
            }
            for j in range(NW):
                items = phase_fill[j]
                nchunks = (4 * j + 4) * NEC
                state = {"i": 0, "t": 0}

                def tick(items=items, nchunks=nchunks, state=state):
                    state["t"] += 1
                    target = len(items) * state["t"] // nchunks
                    while state["i"] < target:
                        items[state["i"]]()
                        state["i"] += 1

                for hp in range(NEC):
                    attention_unit(j, hp, tick)
                while state["i"] < len(items):
                    items[state["i"]]()
                    state["i"] += 1
            for it in out_items(3):
                it()
            if DEBUG:
                t4 = sm.tile([128, 512], F32, tag="dbg4", bufs=1)
                nc.vector.tensor_copy(t4, ctx[(0, 0)])
                nc.sync.dma_start(out=dbg_ctx[:, :], in_=t4)

    nc.compile()
    return nc


def _in_maps(queries, keys, values, Wq, Wk, Wv, Wo):
    import ml_dtypes

    bf16 = ml_dtypes.bfloat16
    scale = np.float32(0.125)  # (DK//H) ** -0.5, exact power of two
    xts = []
    for b in range(B):
        xts.append({
            "xqt": np.ascontiguousarray(
                np.asarray(queries[b], np.float32).T).astype(bf16),
            "xkt": np.ascontiguousarray(
                np.asarray(keys[b], np.float32).T).astype(bf16),
            "xvt": np.ascontiguousarray(
                np.asarray(values[b], np.float32).T).astype(bf16),
        })
    wslices = []
    for g in range(2):
        sl = slice(g * E_LOCAL, (g + 1) * E_LOCAL)
        wslices.append({
            "wq": np.ascontiguousarray(
                np.asarray(Wq[:, sl], np.float32) * scale).astype(bf16),
            "wk": np.ascontiguousarray(
                np.asarray(Wk[:, sl], np.float32)).astype(bf16),
            "wv": np.ascontiguousarray(
                np.asarray(Wv[:, sl], np.float32)).astype(bf16),
            "wo": np.ascontiguousarray(
                np.asarray(Wo[sl, :], np.float32)).astype(bf16),
        })
    in_maps = []
    for c in range(N_CORES):
        b, g = divmod(c, 2)
        m = dict(xts[b])
        m.update(wslices[g])
        in_maps.append(m)
    return in_maps


def kernel(queries, keys, values, mask=None, Wq=None, Wk=None, Wv=None,
           Wo=None, **_ignored):
    from concourse.bass_utils import run_bass_kernel_spmd

    if "nc" not in _cached:
        _cached["nc"] = _build()
    nc = _cached["nc"]

    in_maps = _in_maps(queries, keys, values, Wq, Wk, Wv, Wo)
    res = run_bass_kernel_spmd(nc, in_maps, core_ids=list(range(N_CORES)))
    outs = res.results
    full = np.empty((B, S, HID), np.float32)
    for b in range(B):
        full[b] = outs[2 * b]["out"] + outs[2 * b + 1]["out"]
    return full


def run_traced(inputs, tmpdir=None):
    """Run once with NTFF tracing; returns BassKernelResults."""
    from concourse.bass_utils import run_bass_kernel_spmd

    if "nc" not in _cached:
        _cached["nc"] = _build()
    nc = _cached["nc"]
    in_maps = _in_maps(inputs["queries"], inputs["keys"], inputs["values"],
                       inputs["Wq"], inputs["Wk"], inputs["Wv"], inputs["Wo"])
    return run_bass_kernel_spmd(nc, in_maps, core_ids=list(range(N_CORES)),
                                trace=True, tmpdir=tmpdir)
